# revision 1
# baseline (speedup 1.0000x reference)
"""Trainium2 Bass kernel for nn_ContextRelation_Module (dense_transformer).

Data-parallel over batch: 8 batches -> 8 NeuronCores, one batch each.

Per-core program (B=1 slice):
  x    [512, 16384]  (C_in, H*W)
  q    = relu(bn(W_q2 @ relu(bn(W_q1 @ x))))            [256, 16384]
  k    = relu(bn(W_k2 @ relu(bn(W_k1 @ ctx))))          [256, 19]
  v    = relu(bn(W_v @ ctx))                            [256, 19]
  simT = k^T @ q * (1/16)                               [19, 16384]
  attT = softmax_j(simT)  (no max-subtract: sim in [0, 0.17])
  ctxT = v^T @ attT                                     [256, 16384]
  y    = relu(bn(W_u @ ctxT))                           [512, 16384]

All big matmuls run with float32r operand tag (fp32 bits, fast PE mode).
BN (inference) is folded host-side into per-channel scale/bias applied by
the ScalarE activation (relu(psum*scale + bias)).

Softmax over the 19-entry partition axis is done with PE helpers:
  denom[1,T]  = ones[19,1]^T @ exp_simT           (partition-dim sum)
  bcast[19,T] = ones[1,19]^T @ recip(denom)       (partition broadcast)
  attT        = exp_simT * bcast                  (DVE)

The per-512-column tile pipeline is software-pipelined across five
emission stages (projections+sim | denom | bcast+normalize | ctx | out)
so PE never waits on ACT/DVE softmax latencies.
"""

import numpy as np

import concourse.bacc as bacc
import concourse.bass as bass
import concourse.mybir as mybir
import concourse.tile as tile
from concourse import bass_utils
from concourse.bass import ts
from concourse.masks import make_identity

AFT = mybir.ActivationFunctionType
F32 = mybir.dt.float32

# problem dims (hardcoded per contract)
B = 8
C = 512            # input/output channels
K = 256            # key_channels
H = 128
W = 128
NCTX = 19          # context tokens
NPIX = H * W       # 16384 pixels per batch
CB = C // 128      # 4 partition blocks of C
KB = K // 128      # 2 partition blocks of K
EPS = 1e-5
SOFTMAX_SCALE = K ** -0.5   # 1/16

# tunables
TN = 512                       # free-dim tile (one PSUM bank of fp32)
MM_DT = mybir.dt.float32r      # matmul operand tag for the big GEMMs
X_DMA = "sync"                 # engine for x-in DMA
Y_DMA = "gpsimd"               # engine for y-out DMA


def _build(npix=NPIX, mm_dt=MM_DT, repeat=1):
    """Build + compile the per-core Bass module.

    repeat>1 re-runs the whole pixel loop (same input/output) for
    differential timing: t(R) - t(1) = (R-1) * T_kernel.
    """
    nt = npix // TN
    MMD = mm_dt  # dtype for tensors feeding the big (fp32r-tagged) matmuls
    nc = bacc.Bacc("TRN2", target_bir_lowering=False, debug=False)

    x_d = nc.dram_tensor("x", [C, npix], MMD, kind="ExternalInput").ap()
    ct_d = nc.dram_tensor("ctxt", [C, NCTX], F32, kind="ExternalInput").ap()
    wq1_d = nc.dram_tensor("wq1", [C, K], MMD, kind="ExternalInput").ap()
    wq2_d = nc.dram_tensor("wq2", [K, K], MMD, kind="ExternalInput").ap()
    wk1_d = nc.dram_tensor("wk1", [C, K], F32, kind="ExternalInput").ap()
    wk2_d = nc.dram_tensor("wk2", [K, K], F32, kind="ExternalInput").ap()
    wv_d = nc.dram_tensor("wv", [C, K], F32, kind="ExternalInput").ap()
    wu_d = nc.dram_tensor("wu", [K, C], MMD, kind="ExternalInput").ap()
    sb_names = ["sq1", "bq1", "sq2", "bq2", "sk1", "bk1", "sk2", "bk2",
                "sv", "bv", "su", "bu"]
    sb_d = {}
    for n in sb_names:
        nblk = CB if n in ("su", "bu") else KB
        sb_d[n] = nc.dram_tensor(n, [128, nblk], F32, kind="ExternalInput").ap()
    y_d = nc.dram_tensor("y", [C, npix], F32, kind="ExternalOutput").ap()

    x_v = x_d.rearrange("(c p) n -> p c n", p=128)
    y_v = y_d.rearrange("(c p) n -> p c n", p=128)

    def mmx(out, lhsT, rhs, start, stop):
        nc.tensor.matmul(out, lhsT, rhs, start=start, stop=stop)

    with tile.TileContext(nc) as tc, nc.allow_low_precision(reason="fp32r matmul operands"):
        with (
            tc.tile_pool(name="consts", bufs=1) as consts,
            tc.tile_pool(name="xin", bufs=4) as xin,
            tc.tile_pool(name="yout", bufs=4) as yout,
            tc.tile_pool(name="work", bufs=2) as work,
            tc.tile_pool(name="psB", bufs=3, space="PSUM") as psB,
            tc.tile_pool(name="psS", bufs=2, space="PSUM") as psS,
        ):
            # ---- constants ----
            wq1_sb = consts.tile([128, CB, K], MMD, name="wq1_sb")
            nc.sync.dma_start(out=wq1_sb, in_=wq1_d.rearrange("(c p) m -> p c m", p=128))
            wq2_sb = consts.tile([128, KB, K], MMD, name="wq2_sb")
            nc.sync.dma_start(out=wq2_sb, in_=wq2_d.rearrange("(c p) m -> p c m", p=128))
            wk1_sb = consts.tile([128, CB, K], F32, name="wk1_sb")
            nc.sync.dma_start(out=wk1_sb, in_=wk1_d.rearrange("(c p) m -> p c m", p=128))
            wk2_sb = consts.tile([128, KB, K], F32, name="wk2_sb")
            nc.sync.dma_start(out=wk2_sb, in_=wk2_d.rearrange("(c p) m -> p c m", p=128))
            wv_sb = consts.tile([128, CB, K], F32, name="wv_sb")
            nc.sync.dma_start(out=wv_sb, in_=wv_d.rearrange("(c p) m -> p c m", p=128))
            wu_sb = consts.tile([128, KB, C], MMD, name="wu_sb")
            nc.sync.dma_start(out=wu_sb, in_=wu_d.rearrange("(c p) m -> p c m", p=128))
            sb = {}
            for n in sb_names:
                nblk = CB if n in ("su", "bu") else KB
                t_ = consts.tile([128, nblk], F32, name=f"{n}_sb")
                nc.sync.dma_start(out=t_, in_=sb_d[n])
                sb[n] = t_
            ct_sb = consts.tile([128, CB, NCTX], F32, name="ct_sb")
            nc.sync.dma_start(out=ct_sb, in_=ct_d.rearrange("(c p) m -> p c m", p=128))

            ones19_f = consts.tile([NCTX, 1], F32, name="ones19_f")
            nc.vector.memset(ones19_f, 1.0)
            ones19 = consts.tile([NCTX, 1], MMD, name="ones19")
            nc.vector.tensor_copy(ones19, ones19_f)
            ones1_f = consts.tile([1, NCTX], F32, name="ones1_f")
            nc.vector.memset(ones1_f, 1.0)
            ones1 = consts.tile([1, NCTX], MMD, name="ones1")
            nc.vector.tensor_copy(ones1, ones1_f)
            ident = consts.tile([128, 128], F32, name="ident")
            make_identity(nc, ident)

            # ---- preamble: k, v projections of the 19-token context (fp32) ----
            k1_sb = consts.tile([128, KB, NCTX], F32, name="k1_sb")
            for m in range(KB):
                p = psB.tile([128, NCTX], F32, tag="mm", name="pk1")
                for c in range(CB):
                    nc.tensor.matmul(p, wk1_sb[:, c, ts(m, 128)], ct_sb[:, c, :],
                                     start=(c == 0), stop=(c == CB - 1))
                nc.scalar.activation(k1_sb[:, m, :], p, AFT.Relu,
                                     bias=sb["bk1"][:, m:m + 1], scale=sb["sk1"][:, m:m + 1])
            k2_sb = consts.tile([128, KB, NCTX], MMD, name="k2_sb")
            for m in range(KB):
                p = psB.tile([128, NCTX], F32, tag="mm", name="pk2")
                for c in range(KB):
                    nc.tensor.matmul(p, wk2_sb[:, c, ts(m, 128)], k1_sb[:, c, :],
                                     start=(c == 0), stop=(c == KB - 1))
                nc.scalar.activation(k2_sb[:, m, :], p, AFT.Relu,
                                     bias=sb["bk2"][:, m:m + 1], scale=sb["sk2"][:, m:m + 1])
            v_sb = consts.tile([128, KB, NCTX], F32, name="v_sb")
            for m in range(KB):
                p = psB.tile([128, NCTX], F32, tag="mm", name="pv")
                for c in range(CB):
                    nc.tensor.matmul(p, wv_sb[:, c, ts(m, 128)], ct_sb[:, c, :],
                                     start=(c == 0), stop=(c == CB - 1))
                nc.scalar.activation(v_sb[:, m, :], p, AFT.Relu,
                                     bias=sb["bv"][:, m:m + 1], scale=sb["sv"][:, m:m + 1])
            # vT [19, KB, 128] via PE transpose
            vT_sb = consts.tile([NCTX, KB, 128], MMD, name="vT_sb")
            for m in range(KB):
                p = psB.tile([NCTX, 128], F32, tag="mm", name="pvt")
                nc.tensor.transpose(p, v_sb[:, m, :], ident)
                nc.vector.tensor_copy(vT_sb[:, m, :], p)

            # ---- main loop, software-pipelined in 3 emission stages ----
            state = {}

            def stageA(t):
                xt = xin.tile([128, CB, TN], MMD, tag="xt", name="xt")
                dma_in = nc.sync if X_DMA == "sync" else nc.gpsimd
                dma_in.dma_start(out=xt, in_=x_v[:, :, ts(t, TN)])
                q1 = work.tile([128, KB, TN], MMD, tag="q1", name="q1")
                for m in range(KB):
                    p = psB.tile([128, TN], F32, tag="mm", name="pq1")
                    for c in range(CB):
                        mmx(p, wq1_sb[:, c, ts(m, 128)], xt[:, c, :],
                            c == 0, c == CB - 1)
                    nc.scalar.activation(q1[:, m, :], p, AFT.Relu,
                                         bias=sb["bq1"][:, m:m + 1], scale=sb["sq1"][:, m:m + 1])
                q2 = work.tile([128, KB, TN], MMD, tag="q2", name="q2")
                for m in range(KB):
                    p = psB.tile([128, TN], F32, tag="mm", name="pq2")
                    for c in range(KB):
                        mmx(p, wq2_sb[:, c, ts(m, 128)], q1[:, c, :],
                            c == 0, c == KB - 1)
                    nc.scalar.activation(q2[:, m, :], p, AFT.Relu,
                                         bias=sb["bq2"][:, m:m + 1], scale=sb["sq2"][:, m:m + 1])
                psim = psS.tile([NCTX, TN], F32, tag="s19", name="psim")
                for c in range(KB):
                    mmx(psim, k2_sb[:, c, :], q2[:, c, :], c == 0, c == KB - 1)
                esim = work.tile([NCTX, TN], MMD, tag="esim", name="esim", bufs=3)
                nc.scalar.activation(esim, psim, AFT.Exp, scale=SOFTMAX_SCALE)
                state[t] = {"esim": esim}

            def stageP(t):
                st = state[t]
                ps1 = psS.tile([1, TN], F32, tag="s1", name="ps1", bufs=1)
                mmx(ps1, ones19, st["esim"], True, True)
                recip = work.tile([1, TN], MMD, tag="recip", name="recip")
                nc.vector.reciprocal(recip, ps1)
                st["recip"] = recip

            def stageQ(t):
                st = state[t]
                pbc = psS.tile([NCTX, TN], F32, tag="s19", name="pbc")
                mmx(pbc, ones1, st["recip"], True, True)
                att = work.tile([NCTX, TN], MMD, tag="att", name="att")
                nc.vector.tensor_mul(att, st["esim"], pbc)
                st["att"] = att

            def stageB1(t):
                st = state[t]
                att = st["att"]
                cxt = work.tile([128, KB, TN], MMD, tag="cxt", name="cxt")
                for m in range(KB):
                    p = psB.tile([128, TN], F32, tag="mm", name="pctx")
                    mmx(p, vT_sb[:, m, :], att, True, True)
                    nc.vector.tensor_copy(cxt[:, m, :], p)
                st["cxt"] = cxt

            def stageB2(t):
                st = state.pop(t)
                cxt = st["cxt"]
                yt = yout.tile([128, CB, TN], F32, tag="yt", name="yt")
                for m in range(CB):
                    p = psB.tile([128, TN], F32, tag="pu", name="pu", bufs=2)
                    for c in range(KB):
                        mmx(p, wu_sb[:, c, ts(m, 128)], cxt[:, c, :],
                            c == 0, c == KB - 1)
                    if m % 2 == 0:
                        nc.scalar.activation(yt[:, m, :], p, AFT.Relu,
                                             bias=sb["bu"][:, m:m + 1], scale=sb["su"][:, m:m + 1])
                    else:
                        nc.vector.tensor_scalar(yt[:, m, :], p, sb["su"][:, m:m + 1],
                                                sb["bu"][:, m:m + 1],
                                                mybir.AluOpType.mult, mybir.AluOpType.add)
                        nc.vector.tensor_scalar_max(yt[:, m, :], yt[:, m, :], 0.0)
                dma_out = nc.sync if Y_DMA == "sync" else nc.gpsimd
                dma_out.dma_start(out=y_v[:, :, ts(t, TN)], in_=yt)

            for r in range(repeat):
                for t in range(nt + 4):
                    if t < nt:
                        stageA(t)
                    if 1 <= t <= nt:
                        stageP(t - 1)
                    if 2 <= t <= nt + 1:
                        stageQ(t - 2)
                    if 3 <= t <= nt + 2:
                        stageB1(t - 3)
                    if t >= 4:
                        stageB2(t - 4)

    nc.compile()
    return nc


def _prepare_inputs(inputs, npix=NPIX):
    """Fold BN, transpose weights, shard over batch. Returns list of in_maps."""
    f = np.float32

    def fold(bn, conv_b):
        g, be, m, v = [np.asarray(a, dtype=np.float64) for a in bn]
        s = g / np.sqrt(v + EPS)
        t = be - m * s
        bias = np.asarray(conv_b, dtype=np.float64) * s + t
        return s.astype(f), bias.astype(f)

    def pack(vec):  # [C'] -> [128, C'//128], channel = blk*128 + p
        return np.ascontiguousarray(np.asarray(vec, f).reshape(-1, 128).T)

    sq1, bq1 = fold(inputs["qbn1"], inputs["qb1"])
    sq2, bq2 = fold(inputs["qbn2"], inputs["qb2"])
    sk1, bk1 = fold(inputs["kbn1"], inputs["kb1"])
    sk2, bk2 = fold(inputs["kbn2"], inputs["kb2"])
    sv, bv = fold(inputs["vbn"], inputs["vb"])
    su, bu = fold(inputs["ubn"], inputs["ub"])

    base = {
        "wq1": np.ascontiguousarray(np.asarray(inputs["qW1"], f).T),
        "wq2": np.ascontiguousarray(np.asarray(inputs["qW2"], f).T),
        "wk1": np.ascontiguousarray(np.asarray(inputs["kW1"], f).T),
        "wk2": np.ascontiguousarray(np.asarray(inputs["kW2"], f).T),
        "wv": np.ascontiguousarray(np.asarray(inputs["vW"], f).T),
        "wu": np.ascontiguousarray(np.asarray(inputs["uW"], f).T),
        "sq1": pack(sq1), "bq1": pack(bq1), "sq2": pack(sq2), "bq2": pack(bq2),
        "sk1": pack(sk1), "bk1": pack(bk1), "sk2": pack(sk2), "bk2": pack(bk2),
        "sv": pack(sv), "bv": pack(bv), "su": pack(su), "bu": pack(bu),
    }
    x = np.asarray(inputs["x"], f)
    ctx = np.asarray(inputs["context"], f)
    in_maps = []
    for b_i in range(x.shape[0]):
        m = dict(base)
        m["x"] = np.ascontiguousarray(x[b_i].reshape(C, -1)[:, :npix])
        m["ctxt"] = np.ascontiguousarray(ctx[b_i].reshape(C, NCTX))
        in_maps.append(m)
    return in_maps


_NC_CACHE = {}


def _get_nc(npix=NPIX):
    key = (npix, str(MM_DT), TN)
    if key not in _NC_CACHE:
        _NC_CACHE[key] = _build(npix)
    return _NC_CACHE[key]


def run(inputs, trace=False, **kwargs):
    """Run on 8 cores; returns (y [8,512,128,128], BassKernelResults)."""
    nc = _get_nc()
    in_maps = _prepare_inputs(inputs)
    res = bass_utils.run_bass_kernel_spmd(
        nc, in_maps, core_ids=list(range(B)), trace=trace, **kwargs)
    y = np.stack([res.results[b]["y"].reshape(C, H, W) for b in range(B)])
    return y.astype(np.float32), res


def kernel(**inputs):
    y, _ = run(inputs)
    return y



# revision 3
# speedup vs baseline: 1.3606x; 1.3606x over previous
"""Trainium2 Bass kernel for nn_ContextRelation_Module (dense_transformer).

Data-parallel over batch: 8 batches -> 8 NeuronCores, one batch each.

Per-core program (B=1 slice), algebraically restructured vs the module:

  x     [512, 16384]   (C_in, H*W), bf16
  q1'   = relu(Wq1 @ x + bq1/s1)                     [256, T]  (s1 folded into Wq2)
  q2'   = relu(Wq2' @ q1' + bq2/s2)                  [256, T]  (s2 folded into k2)
  sim   = k2'^T @ q2'                                [19, T]
  esim  = exp(sim / 16)          (ACT, [19,T])
  esimT = PE-transpose(esim)     -> [pix, 19] blocks (softmax on the FREE axis)
  att   = esimT * recip(rowsum)  (DVE, per-partition scalars -- cheap)
  attT  = PE-transpose(att)      -> [19, T]
  y     = relu(uvT^T @ attT + bu)                    [512, T]

where uvT = (Wu' @ v)^T [19, 512] is computed once in the preamble --
u-projection folded through the (linear) attention-weighted sum, removing
the [256,T] ctx intermediate entirely.  All BN scales are folded into the
next layer's weights (host- or preamble-side) so every PSUM drain is a
single instruction: relu(psum + bias) on either ACT or DVE.

Softmax normalization happens in the [pix(partition), 19(free)] layout, so
the row-sum/reciprocal/scale are tiny per-partition DVE ops (the [1,T]
partition-reduce + reciprocal + broadcast of the naive layout cost ~4.3us
of DVE time per tile on real HW -- reciprocal over a 1-partition AP is
serial).

Big-GEMM operands (x, Wq1, Wq2', k2', uvT, attT) are bf16; accumulation is
fp32 in PSUM; the k/v/uv preamble stays fp32.
"""

import numpy as np

import concourse.bacc as bacc
import concourse.bass as bass
import concourse.mybir as mybir
import concourse.tile as tile
from concourse import bass_utils
from concourse.bass import ts
from concourse.masks import make_identity

AFT = mybir.ActivationFunctionType
ALU = mybir.AluOpType
F32 = mybir.dt.float32
BF16 = mybir.dt.bfloat16

# problem dims (hardcoded per contract)
B = 8
C = 512            # input/output channels
K = 256            # key_channels
H = 128
W = 128
NCTX = 19          # context tokens
NPIX = H * W       # 16384 pixels per batch
CB = C // 128      # 4 partition blocks of C
KB = K // 128      # 2 partition blocks of K
EPS = 1e-5
SOFTMAX_SCALE = K ** -0.5   # 1/16

# tunables
TN = 512                       # free-dim tile (one PSUM bank of fp32)
PB = TN // 128                 # pixel blocks per tile (for the transposes)


def _build(npix=NPIX, repeat=1):
    """Build + compile the per-core Bass module.

    repeat>1 re-runs the whole pixel loop (same input/output) for
    differential timing.
    """
    nt = npix // TN
    nc = bacc.Bacc("TRN2", target_bir_lowering=False, debug=False)

    x_d = nc.dram_tensor("x", [C, npix], BF16, kind="ExternalInput").ap()
    ct_d = nc.dram_tensor("ctxt", [C, NCTX], F32, kind="ExternalInput").ap()
    wq1_d = nc.dram_tensor("wq1", [C, K], BF16, kind="ExternalInput").ap()
    wq2_d = nc.dram_tensor("wq2", [K, K], BF16, kind="ExternalInput").ap()
    wk1_d = nc.dram_tensor("wk1", [C, K], F32, kind="ExternalInput").ap()
    wk2_d = nc.dram_tensor("wk2", [K, K], F32, kind="ExternalInput").ap()
    wv_d = nc.dram_tensor("wv", [C, K], F32, kind="ExternalInput").ap()
    wu_d = nc.dram_tensor("wu", [K, C], F32, kind="ExternalInput").ap()
    # packed per-channel vectors: [128, nblk], channel = blk*128 + p
    sb_names = ["bq1", "bq2", "sk1", "bk1", "sk2", "bk2", "sq2", "sv", "bv", "bu"]
    sb_d = {}
    for n in sb_names:
        nblk = CB if n == "bu" else KB
        sb_d[n] = nc.dram_tensor(n, [128, nblk], F32, kind="ExternalInput").ap()
    y_d = nc.dram_tensor("y", [C, npix], F32, kind="ExternalOutput").ap()

    x_v = x_d.rearrange("(c p) n -> p c n", p=128)
    y_v = y_d.rearrange("(c p) n -> p c n", p=128)

    with tile.TileContext(nc) as tc, nc.allow_low_precision(reason="bf16 matmul operands"):
        with (
            tc.tile_pool(name="consts", bufs=1) as consts,
            tc.tile_pool(name="xin", bufs=3) as xin,
            tc.tile_pool(name="yout", bufs=3) as yout,
            tc.tile_pool(name="work", bufs=2) as work,
            tc.tile_pool(name="psA", bufs=2, space="PSUM") as psA,      # q1/q2 [128,512]
            tc.tile_pool(name="psS", bufs=2, space="PSUM") as psS,      # sim [19,512]
            tc.tile_pool(name="psT1", bufs=1, space="PSUM") as psT1,    # esimT [128,PB,19]
            tc.tile_pool(name="psT2", bufs=1, space="PSUM") as psT2,    # attT [19,512]
            tc.tile_pool(name="psY", bufs=2, space="PSUM") as psY,      # y [128,512]
        ):
            # ---- constants ----
            wq1_sb = consts.tile([128, CB, K], BF16, name="wq1_sb")
            nc.sync.dma_start(out=wq1_sb, in_=wq1_d.rearrange("(c p) m -> p c m", p=128))
            wq2_sb = consts.tile([128, KB, K], BF16, name="wq2_sb")
            nc.sync.dma_start(out=wq2_sb, in_=wq2_d.rearrange("(c p) m -> p c m", p=128))
            wk1_sb = consts.tile([128, CB, K], F32, name="wk1_sb")
            nc.sync.dma_start(out=wk1_sb, in_=wk1_d.rearrange("(c p) m -> p c m", p=128))
            wk2_sb = consts.tile([128, KB, K], F32, name="wk2_sb")
            nc.sync.dma_start(out=wk2_sb, in_=wk2_d.rearrange("(c p) m -> p c m", p=128))
            wv_sb = consts.tile([128, CB, K], F32, name="wv_sb")
            nc.sync.dma_start(out=wv_sb, in_=wv_d.rearrange("(c p) m -> p c m", p=128))
            wu_sb = consts.tile([128, KB, C], F32, name="wu_sb")
            nc.sync.dma_start(out=wu_sb, in_=wu_d.rearrange("(c p) m -> p c m", p=128))
            sb = {}
            for n in sb_names:
                nblk = CB if n == "bu" else KB
                t_ = consts.tile([128, nblk], F32, name=f"{n}_sb")
                nc.sync.dma_start(out=t_, in_=sb_d[n])
                sb[n] = t_
            ct_sb = consts.tile([128, CB, NCTX], F32, name="ct_sb")
            nc.sync.dma_start(out=ct_sb, in_=ct_d.rearrange("(c p) m -> p c m", p=128))

            ident = consts.tile([128, 128], F32, name="ident")
            make_identity(nc, ident)

            # ---- preamble: k2' (bf16, s_q2-scaled), uvT (bf16) ----
            k1_sb = consts.tile([128, KB, NCTX], F32, name="k1_sb")
            for m in range(KB):
                pf = psA.tile([128, TN], F32, tag="mm", name="pk1")
                p = pf[:, :NCTX]
                for c in range(CB):
                    nc.tensor.matmul(p, wk1_sb[:, c, ts(m, 128)], ct_sb[:, c, :],
                                     start=(c == 0), stop=(c == CB - 1))
                nc.scalar.activation(k1_sb[:, m, :], p, AFT.Relu,
                                     bias=sb["bk1"][:, m:m + 1], scale=sb["sk1"][:, m:m + 1])
            k2_sb = consts.tile([128, KB, NCTX], BF16, name="k2_sb")
            for m in range(KB):
                pf = psA.tile([128, TN], F32, tag="mm", name="pk2")
                p = pf[:, :NCTX]
                for c in range(KB):
                    nc.tensor.matmul(p, wk2_sb[:, c, ts(m, 128)], k1_sb[:, c, :],
                                     start=(c == 0), stop=(c == KB - 1))
                # relu(s*psum+b) then * s_q2 (folded from the q2 BN scale)
                kf = consts.tile([128, NCTX], F32, name=f"k2f{m}")
                nc.scalar.activation(kf, p, AFT.Relu,
                                     bias=sb["bk2"][:, m:m + 1], scale=sb["sk2"][:, m:m + 1])
                nc.vector.tensor_scalar_mul(k2_sb[:, m, :], kf, sb["sq2"][:, m:m + 1])
            v_sb = consts.tile([128, KB, NCTX], F32, name="v_sb")
            for m in range(KB):
                pf = psA.tile([128, TN], F32, tag="mm", name="pv")
                p = pf[:, :NCTX]
                for c in range(CB):
                    nc.tensor.matmul(p, wv_sb[:, c, ts(m, 128)], ct_sb[:, c, :],
                                     start=(c == 0), stop=(c == CB - 1))
                nc.scalar.activation(v_sb[:, m, :], p, AFT.Relu,
                                     bias=sb["bv"][:, m:m + 1], scale=sb["sv"][:, m:m + 1])
            # uvT [19, C] = v^T @ Wu'^T  (s_u already folded into wu host-side)
            puv = psS.tile([NCTX, TN], F32, tag="sim", name="puv")
            for c in range(KB):
                nc.tensor.matmul(puv, v_sb[:, c, :], wu_sb[:, c, :],
                                 start=(c == 0), stop=(c == KB - 1))
            uvT_sb = consts.tile([NCTX, C], BF16, name="uvT_sb")
            nc.scalar.activation(uvT_sb, puv, AFT.Copy)

            # ---- main loop, software-pipelined ----
            state = {}

            def s0a(t):  # x dma + q1 m0
                xt = xin.tile([128, CB, TN], BF16, tag="xt", name="xt")
                nc.sync.dma_start(out=xt, in_=x_v[:, :, ts(t, TN)])
                q1 = work.tile([128, KB, TN], BF16, tag="q1", name="q1")
                p = psA.tile([128, TN], F32, tag="mm", name="pq1a")
                for c in range(CB):
                    nc.tensor.matmul(p, wq1_sb[:, c, ts(0, 128)], xt[:, c, :],
                                     start=(c == 0), stop=(c == CB - 1))
                nc.scalar.activation(q1[:, 0, :], p, AFT.Relu, bias=sb["bq1"][:, 0:1])
                state[t] = {"xt": xt, "q1": q1}

            def s0b(t):  # q1 m1
                st = state[t]
                p = psA.tile([128, TN], F32, tag="mm", name="pq1b")
                for c in range(CB):
                    nc.tensor.matmul(p, wq1_sb[:, c, ts(1, 128)], st["xt"][:, c, :],
                                     start=(c == 0), stop=(c == CB - 1))
                nc.vector.tensor_scalar(q1b := st["q1"][:, 1, :], p,
                                        sb["bq1"][:, 1:2], 0.0, ALU.add, ALU.max)

            def s0c(t):  # q2 both blocks
                st = state[t]
                q2 = work.tile([128, KB, TN], BF16, tag="q2", name="q2")
                for m in range(KB):
                    p = psA.tile([128, TN], F32, tag="mm", name="pq2")
                    for c in range(KB):
                        nc.tensor.matmul(p, wq2_sb[:, c, ts(m, 128)], st["q1"][:, c, :],
                                         start=(c == 0), stop=(c == KB - 1))
                    if m == 0:
                        nc.scalar.activation(q2[:, m, :], p, AFT.Relu, bias=sb["bq2"][:, m:m + 1])
                    else:
                        nc.vector.tensor_scalar(q2[:, m, :], p,
                                                sb["bq2"][:, m:m + 1], 0.0, ALU.add, ALU.max)
                st["q2"] = q2

            def s0d(t):  # sim [19, TN]
                st = state[t]
                psim = psS.tile([NCTX, TN], F32, tag="sim", name="psim")
                for c in range(KB):
                    nc.tensor.matmul(psim, k2_sb[:, c, :], st["q2"][:, c, :],
                                     start=(c == 0), stop=(c == KB - 1))
                st["psim"] = psim

            def s1(t):  # softmax in [pix, 19] layout
                st = state[t]
                esim = work.tile([NCTX, TN], F32, tag="esim", name="esim")
                nc.scalar.activation(esim, st["psim"], AFT.Exp, scale=SOFTMAX_SCALE)
                pT1 = psT1.tile([128, PB, NCTX], F32, tag="t1", name="pT1")
                for b in range(PB):
                    nc.tensor.transpose(pT1[:, b, :], esim[:, ts(b, 128)], ident[0:NCTX, 0:NCTX])
                sums = work.tile([128, PB, 1], F32, tag="sums", name="sums")
                nc.vector.reduce_sum(sums, pT1, axis=mybir.AxisListType.X)
                recip = work.tile([128, PB], F32, tag="recip", name="recip")
                nc.vector.reciprocal(recip, sums[:, :, 0])
                att = work.tile([128, PB, NCTX], F32, tag="att", name="att")
                for b in range(PB):
                    nc.vector.tensor_scalar_mul(att[:, b, :], pT1[:, b, :], recip[:, b:b + 1])
                st["att"] = att

            def s2(t):  # transpose back to [19, TN], cast bf16
                st = state.get(t)
                pT2 = psT2.tile([NCTX, TN], F32, tag="t2", name="pT2")
                for b in range(PB):
                    nc.tensor.transpose(pT2[:, ts(b, 128)], st["att"][:, b, :], ident)
                attT = work.tile([NCTX, TN], BF16, tag="attT", name="attT")
                nc.scalar.activation(attT, pT2, AFT.Copy)
                st["attT"] = attT

            def s3(t, ms):  # y blocks
                st = state[t]
                if "yt" not in st:
                    st["yt"] = yout.tile([128, CB, TN], F32, tag="yt", name="yt")
                yt = st["yt"]
                for m in ms:
                    p = psY.tile([128, TN], F32, tag="y", name="py")
                    nc.tensor.matmul(p, uvT_sb[:, ts(m, 128)], st["attT"],
                                     start=True, stop=True)
                    if m % 2 == 0:
                        nc.scalar.activation(yt[:, m, :], p, AFT.Relu, bias=sb["bu"][:, m:m + 1])
                    else:
                        nc.vector.tensor_scalar(yt[:, m, :], p,
                                                sb["bu"][:, m:m + 1], 0.0, ALU.add, ALU.max)
                if ms[-1] == CB - 1:
                    nc.gpsimd.dma_start(out=y_v[:, :, ts(t, TN)], in_=yt)
                    state.pop(t)

            for r in range(repeat):
                for t in range(nt + 3):
                    if t >= 3:
                        s3(t - 3, [0, 1])
                    if t < nt:
                        s0a(t)
                    if t >= 3:
                        s3(t - 3, [2, 3])
                    if t < nt:
                        s0b(t)
                    if 1 <= t <= nt:
                        s1(t - 1)
                    if t < nt:
                        s0c(t)
                    if 2 <= t <= nt + 1:
                        s2(t - 2)
                    if t < nt:
                        s0d(t)

    nc.compile()
    return nc


def _prepare_inputs(inputs, npix=NPIX):
    """Fold BN into weights/biases, transpose, shard over batch."""
    import ml_dtypes
    f = np.float32
    bf = ml_dtypes.bfloat16

    def fold(bn, conv_b):
        g, be, m, v = [np.asarray(a, dtype=np.float64) for a in bn]
        s = g / np.sqrt(v + EPS)
        t = be - m * s
        bias = np.asarray(conv_b, dtype=np.float64) * s + t
        return s, bias

    def pack(vec):  # [C'] -> [128, C'//128], channel = blk*128 + p
        return np.ascontiguousarray(np.asarray(vec, f).reshape(-1, 128).T)

    s1, b1 = fold(inputs["qbn1"], inputs["qb1"])
    s2, b2 = fold(inputs["qbn2"], inputs["qb2"])
    sk1, bk1 = fold(inputs["kbn1"], inputs["kb1"])
    sk2, bk2 = fold(inputs["kbn2"], inputs["kb2"])
    sv, bv = fold(inputs["vbn"], inputs["vb"])
    su, bu = fold(inputs["ubn"], inputs["ub"])

    qW2 = np.asarray(inputs["qW2"], np.float64)
    uW = np.asarray(inputs["uW"], np.float64)

    base = {
        "wq1": np.ascontiguousarray(np.asarray(inputs["qW1"], f).T.astype(bf)),
        # fold s1 into Wq2 columns (input-channel scaling); transposed layout [in, out]
        "wq2": np.ascontiguousarray((qW2 * s1[None, :]).T.astype(f).astype(bf)),
        "wk1": np.ascontiguousarray(np.asarray(inputs["kW1"], f).T),
        "wk2": np.ascontiguousarray(np.asarray(inputs["kW2"], f).T),
        "wv": np.ascontiguousarray(np.asarray(inputs["vW"], f).T),
        # fold s_u into Wu rows (output-channel scaling); transposed layout [in, out]
        "wu": np.ascontiguousarray((uW * su[:, None]).T.astype(f)),
        "bq1": pack(b1 / s1), "bq2": pack(b2 / s2),
        "sk1": pack(sk1), "bk1": pack(bk1), "sk2": pack(sk2), "bk2": pack(bk2),
        "sq2": pack(s2), "sv": pack(sv), "bv": pack(bv), "bu": pack(bu),
    }
    x = np.asarray(inputs["x"], f)
    ctx = np.asarray(inputs["context"], f)
    in_maps = []
    for b_i in range(x.shape[0]):
        m = dict(base)
        m["x"] = np.ascontiguousarray(x[b_i].reshape(C, -1)[:, :npix].astype(bf))
        m["ctxt"] = np.ascontiguousarray(ctx[b_i].reshape(C, NCTX))
        in_maps.append(m)
    return in_maps


_NC_CACHE = {}


def _get_nc(npix=NPIX):
    key = (npix, TN)
    if key not in _NC_CACHE:
        _NC_CACHE[key] = _build(npix)
    return _NC_CACHE[key]


def run(inputs, trace=False, **kwargs):
    """Run on 8 cores; returns (y [8,512,128,128], BassKernelResults)."""
    nc = _get_nc()
    in_maps = _prepare_inputs(inputs)
    res = bass_utils.run_bass_kernel_spmd(
        nc, in_maps, core_ids=list(range(B)), trace=trace, **kwargs)
    y = np.stack([res.results[b]["y"].reshape(C, H, W) for b in range(B)])
    return y.astype(np.float32), res


def kernel(**inputs):
    y, _ = run(inputs)
    return y
